# revision 34
# baseline (speedup 1.0000x reference)
"""BERT self-attention (B=4, S=2048, H=768, 12 heads x d=64) on 8 Trainium2
NeuronCores.

Sharding: core c handles batch b = c//2 and head group hg = c%2 (6 heads).
No cross-core communication; the host scatters inputs and gathers the output.

v3 design (vs v2's 356us):
  The scalar engine's softmax exp is the hard floor: exp of all scores
  (25.2M elems/core) at 1 elem/cyc/partition @1.2GHz = 164us, plus a fixed
  ~293ns per ACTIVATE instruction that does NOT pipeline across instructions
  (probed).  PSUM (8 banks) bounds the ACTIVATE group size: scores ring gets
  6 banks (2 groups x 3 banks, double-buffered), ctx accumulators 2 banks.
  So the plan keeps ACT 100% busy on ~1536-elem groups (~201us) and hides
  everything else under it:
  - scores matmuls are K=64; the two heads of a pair use PE row-groups
    0:64 / 64:128 and run CONCURRENTLY (probed 2x; auto tile_position).
    PE total ~405k cyc = ~169us @2.4GHz < ACT.
  - chunk = (head pair, 512-query quarter): 12 chunks x 16 key-tiles.
    Per jt: 2 concurrent scores matmuls -> 2 ring banks; ACT exps each
    3-bank group into fp16 `ex`.
  - ctx is TRANSPOSED: stationary = ex [128 keys, 128 queries], moving =
    v [128 keys, 66] (cols 64:66 = ones), out = [128 queries, 66] psum.
    Queries land on psum PARTITIONS, so sum(exp) is column 64 and the
    softmax divide is a per-partition-scalar op: reciprocal [128,1] +
    tensor_scalar_mul — no cross-partition broadcast, no DRAM roundtrip.
    The 4 query-tile accumulators of a head share one psum bank; only the
    very first matmul per bank uses start=True (start clears has_written
    for the WHOLE bank — probed), everything else start=False.
  - projections q/k/v are injected into the ring as extra slots (the ACT
    stream skips them; DVE casts them to fp16 sbuf).
  - fp16 everywhere (same PE/DVE speed as bf16, 8x finer mantissa).

Per-core layouts (SBUF [128 partitions x free]):
  xT   [128, KC, 2048] fp16   x[b].T by contraction chunk
  wq/wk/wv [128, KC, 384] fp16
  qT/kT [128, 3, 2048] fp16   per head-pair stacked d-dims (even head p0:64,
                              odd head p64:128)
  v    [128, 16, 6, 66] fp16  token-major v; cols 64:66 of each head = ones
  ring psum: 2 x [128, 3, 512] f32 (6 banks); ctx psum: 2 x [128, 4, 66] (2)

Known framework pitfall (verified in BIR): a DMA reader of a tile waits on
only ONE prior writer's semaphore — never give a DMA-read tile multiple
writers.  (The v3 DRAM z-route did, and raced on first execution.)
"""
import os
from collections import deque

import numpy as np

if not os.environ.get("KERNEL_TRACE"):
    os.environ.setdefault("BASS_NEVER_TRACE", "1")

import concourse.bass as bass
import concourse.mybir as mybir
import concourse.tile as tile
from concourse import bacc
from concourse.bass import ts
from concourse.bass_utils import run_bass_kernel_spmd

F32 = mybir.dt.float32
F16 = mybir.dt.float16

HIDDEN = 768
N_HEADS = 12
HEAD_DIM = 64
B = 4
S = 2048
HPC = 6           # heads per core
NPAIR = HPC // 2  # 3 head pairs
NJ = S // 128     # 16 key tiles per chunk
NQT = 4           # query quarters (512 q each)
NCHUNK = NPAIR * NQT  # 12
CTX_LAG = 28      # ctx pop backlog in ring slots (~9 groups)

_cache = {}
last_results = None


def _build(use_mask: bool, use_bias: bool):
    KC = 7 if use_bias else 6
    nc = bacc.Bacc("TRN2", target_bir_lowering=False, debug=False, num_devices=8)

    xT_d = nc.dram_tensor("xT", [KC * 128, S], F16, kind="ExternalInput")
    wq_d = nc.dram_tensor("wq", [KC * 128, HPC * HEAD_DIM], F16, kind="ExternalInput")
    wk_d = nc.dram_tensor("wk", [KC * 128, HPC * HEAD_DIM], F16, kind="ExternalInput")
    wv_d = nc.dram_tensor("wv", [KC * 128, HPC * HEAD_DIM], F16, kind="ExternalInput")
    if use_mask:
        em_d = nc.dram_tensor("em", [128, NJ], F32, kind="ExternalInput")
    out_d = nc.dram_tensor("out", [HPC, S, HEAD_DIM], F32, kind="ExternalOutput")

    with tile.TileContext(nc) as tc:
        with (
            tc.tile_pool(name="const", bufs=1) as cpool_,
            tc.tile_pool(name="big", bufs=1) as big,
            tc.tile_pool(name="ex", bufs=13) as expool,
            tc.tile_pool(name="zr", bufs=6) as zpool,
            tc.tile_pool(name="oo", bufs=4) as opool,
            tc.tile_pool(name="pg", bufs=2, space="PSUM") as gpool,
            tc.tile_pool(name="pc", bufs=2, space="PSUM") as cxpool,
        ):
            if use_mask:
                em = cpool_.tile([128, NJ], F32)
                nc.sync.dma_start(em[:], em_d[:])

            # xT as four separate 512-token stripe tiles: the framework's
            # reader deps are effectively tile-granular for DMA writers, so
            # one xT tile written by 4 stripe DMAs would gate the first
            # projection on the LAST stripe.
            xTs = [big.tile([128, KC, 512], F16, name=f"xT{tt}")
                   for tt in range(4)]
            wq = big.tile([128, KC, HPC * HEAD_DIM], F16)
            wk = big.tile([128, KC, HPC * HEAD_DIM], F16)
            wv = big.tile([128, KC, HPC * HEAD_DIM], F16)
            # per-pair tiles: reader deps are tile-granular, so a single
            # qT/kT tile would make pair-p scores falsely wait on later
            # pairs' projection casts
            qTs = [big.tile([128, S], F16, name=f"qT{p}")
                   for p in range(NPAIR)]
            kTs = [big.tile([128, S], F16, name=f"kT{p}")
                   for p in range(NPAIR)]
            v = big.tile([128, NJ, HPC, 66], F16)
            wsrc = cpool_.tile([32, 512], F16)

            # input DMA.  The HW DGE executes queued DMAs one at a time
            # (~0.6us each observed), so consolidate into few big transfers
            # and split across the two HWDGE engines (SP + ACT queues).
            # x goes in TOKEN-major stripes (each 512-token stripe carries
            # all contraction chunks) so the first k/q projection tiles are
            # gated on only ~1/4 of x.
            wdview = (wk_d, wq_d, wv_d)

            def w3(w, d, lo, hi, eng):
                eng.dma_start(
                    w[:, :, lo:hi],
                    d[:].rearrange("(c p) n -> p c n", p=128)[:, :, lo:hi])

            # ACT's HWDGE is NOT used for inputs: a dma trigger occupies
            # the issuing engine's queue for the whole transfer, which would
            # push the first exp out by ~15us.  gpsimd (idle) issues via
            # SWDGE instead.
            def xstripe(tt, eng):
                eng.dma_start(
                    xTs[tt][:],
                    xT_d[:].rearrange("(c p) n -> p c n", p=128)[:, :, ts(tt, 512)])

            # priority: the first scores need ONLY stripe0 + wk/wq pair-0
            # columns; everything else queues behind so it does not steal
            # HBM bandwidth from the critical transfers.
            xstripe(0, nc.sync)
            w3(wk, wk_d, 0, 128, nc.gpsimd)
            w3(wq, wq_d, 0, 128, nc.gpsimd)
            xstripe(1, nc.sync)
            xstripe(2, nc.sync)
            xstripe(3, nc.sync)
            w3(wv, wv_d, 0, 384, nc.gpsimd)
            w3(wk, wk_d, 128, 384, nc.sync)
            w3(wq, wq_d, 128, 384, nc.gpsimd)

            # warm the PE p-state and trigger the exp ACT-table load
            # (~2.7us) immediately, before anything else queues.
            nc.vector.memset(wsrc[:], 0.125)
            dummy = cpool_.tile([32, 32], F16)
            for wb in range(2):
                warm = cxpool.tile([128, 4, 66], F32, tag="c",
                                   name=f"warm{wb}")
                wf = warm[:].rearrange("p a n -> p (a n)")
                for i in range(7):
                    nc.tensor.matmul(wf, wsrc[:, 0:128], wsrc[:, 0:264],
                                     start=True, stop=True)
                    if wb == 0 and i == 1:
                        nc.scalar.activation(dummy[:], warm[0:32, 0, 0:32],
                                             mybir.ActivationFunctionType.Exp,
                                             scale=0.125)

            nc.vector.memset(v[:, :, :, HEAD_DIM:66], 1.0)

            # ---- event stream ----
            # ("s", chunk, jt, h) scores slot; ("q"/"k", pair, tt) or
            # ("v", jt) projection slot.  Chunk c = pair*4 + quarter.
            def chunk_events(c):
                ev = [("s", c, jt, h) for jt in range(NJ) for h in (0, 1)]
                inj = []
                if c == 0:
                    inj = [("k", 0, 1), ("k", 0, 2), ("k", 0, 3),
                           ("q", 0, 1)] + [("v", j) for j in range(8)]
                elif c == 1:
                    inj = [("q", 0, 2)] + [("v", j) for j in range(8, NJ)]
                elif c == 2:
                    inj = [("q", 0, 3), ("k", 1, 0), ("k", 1, 1)]
                elif c == 3:
                    inj = [("k", 1, 2), ("k", 1, 3), ("q", 1, 0)]
                elif c == 5:
                    inj = [("q", 1, 2), ("k", 2, 0)]
                elif c == 6:
                    inj = [("q", 1, 3), ("k", 2, 1), ("k", 2, 2)]
                elif c == 7:
                    inj = [("q", 2, 0), ("k", 2, 3)]
                elif c < 11:
                    inj = [("q", *divmod(c + 1, 4))]
                # k/v tiles front-packed (upcoming scores / the lagging
                # chunk's ctx need them soon); q spread evenly
                out = list(ev)
                vs = [e for e in inj if e[0] != "q"]
                rest = [e for e in inj if e[0] == "q"]
                for i, e in enumerate(vs):
                    out.insert(min(1 + 3 * i, len(out)), e)
                n = len(rest)
                for i, e in enumerate(rest):
                    pos = (i + 1) * (len(out) + 1) // (n + 1)
                    out.insert(min(pos, len(out)), e)
                return out

            events = [("k", 0, 0), ("q", 0, 0)]
            for c in range(NCHUNK):
                events.extend(chunk_events(c))

            # nudge proj slots off group-middle positions so ACT runs split
            # as little as possible.  Only ever move a proj EARLIER: moving
            # one later can put it behind a scores event that reads its
            # output, and the tile framework orders by emission — the scores
            # matmul would read uninitialized SBUF (caught by CoreSim as
            # exactly that; on hardware it poisons only the FIRST execution
            # because later runs see the previous run's identical values).
            for i in range(1, len(events)):
                if (i % 3 == 1 and events[i][0] != "s"
                        and events[i - 1][0] == "s"):
                    events[i], events[i - 1] = events[i - 1], events[i]

            # Projection chains are queued as single-matmul closures and
            # drained a couple per scores fill, so a 6-matmul contraction
            # chain never runs as one PE burst that starves the ACT ring.
            # Correctness: a chain is force-flushed before emitting any
            # scores fill / ctx matmul that reads its output (the tile
            # framework orders by emission, so a consumer emitted before
            # its producer would read uninitialized SBUF).
            proj_queue = deque()   # (depkey, closure)
            chain_left = {}        # depkey -> closures still queued

            def enqueue_proj(depkey, fns):
                for f in fns:
                    proj_queue.append((depkey, f))
                chain_left[depkey] = chain_left.get(depkey, 0) + len(fns)

            def drain_proj(n=None, need=None):
                if need is not None:
                    # a needed chain must already be enqueued: the flush of a
                    # never-enqueued chain would silently no-op and the
                    # consumer would read uninitialized SBUF
                    assert need in chain_left, f"proj {need} not yet emitted"
                while proj_queue:
                    if need is not None and chain_left.get(need, 0) == 0:
                        return
                    if need is None and n is not None and n <= 0:
                        return
                    k2, f = proj_queue.popleft()
                    f()
                    chain_left[k2] -= 1
                    if n is not None:
                        n -= 1

            def emit_fill(g, pos, ev):
                kind = ev[0]
                if kind == "s":
                    _, c, jt, h = ev
                    p, qtr = divmod(c, 4)
                    drain_proj(need=("k", p, jt // 4))
                    drain_proj(need=("q", p, qtr))
                    po = 64 * h
                    nc.tensor.matmul(
                        g[:, pos, :], kTs[p][po:po + 64, ts(jt, 128)],
                        qTs[p][po:po + 64, ts(qtr, 512)],
                        start=True, stop=True,
                    )
                    drain_proj(n=3)
                elif kind == "v":
                    _, jt = ev

                    def vmm(c_, g=g, pos=pos, jt=jt):
                        nc.tensor.matmul(
                            g[:, pos, 0:HPC * HEAD_DIM],
                            xTs[jt // 4][:, c_, ts(jt % 4, 128)],
                            wv[:, c_, :],
                            start=(c_ == 0), stop=(c_ == KC - 1),
                        )

                    def vcast(g=g, pos=pos, jt=jt):
                        nc.vector.tensor_copy(
                            v[:, jt, :, 0:HEAD_DIM],
                            g[:, pos, 0:HPC * HEAD_DIM].rearrange(
                                "p (h e) -> p h e", h=HPC),
                        )  # cols 64:66 stay the memset ones

                    enqueue_proj(("v", jt),
                                 [(lambda c_=c_: vmm(c_)) for c_ in range(KC)]
                                 + [vcast])
                else:
                    _, p, tt = ev
                    w_, dst = (wq, qTs[p]) if kind == "q" else (wk, kTs[p])

                    def pmm(c_, g=g, pos=pos, w_=w_, p=p, tt=tt):
                        nc.tensor.matmul(
                            g[:, pos, :], w_[:, c_, ts(p, 128)],
                            xTs[tt][:, c_, :],
                            start=(c_ == 0), stop=(c_ == KC - 1),
                        )

                    def pcast(g=g, pos=pos, dst=dst, tt=tt):
                        nc.vector.tensor_copy(dst[:, ts(tt, 512)],
                                              g[:, pos, :])

                    enqueue_proj((kind, p, tt),
                                 [(lambda c_=c_: pmm(c_)) for c_ in range(KC)]
                                 + [pcast])

            pending = deque()   # (ex, pos, c, jt, h)
            cxt = {}            # (c, h) -> psum tile [128, 4, 66]
            remaining = {c: 2 * NJ for c in range(NCHUNK)}

            def finalize(c):
                p, qtr = divmod(c, 4)
                for h in (0, 1):
                    cx = cxt.pop((c, h))
                    o = opool.tile([128, NQT, HEAD_DIM], F32, tag="o",
                                   name=f"o{c}_{h}")
                    for qt in range(NQT):
                        zrec = zpool.tile([128, 1], F32, tag="zrec",
                                          name=f"zrec{c}_{h}{qt}")
                        nc.vector.reciprocal(zrec[:], cx[:, qt, 64:65])
                        nc.vector.tensor_scalar_mul(
                            o[:, qt, :], cx[:, qt, 0:HEAD_DIM], zrec[:, 0:1])
                        nc.sync.dma_start(
                            out_d[2 * p + h,
                                  qtr * 512 + qt * 128:qtr * 512 + (qt + 1) * 128,
                                  :],
                            o[:, qt, :])

            def pop_ctx():
                ex, pos, c, jt, h = pending.popleft()
                p = c // 4
                drain_proj(need=("v", jt))
                key = (c, h)
                if key not in cxt:
                    cxt[key] = cxpool.tile([128, NQT, 66], F32, tag="c",
                                           name=f"cx{c}_{h}")
                cx = cxt[key]
                for qt in range(NQT):
                    nc.tensor.matmul(
                        cx[:, qt, :], ex[:, pos, ts(qt, 128)],
                        v[:, jt, 2 * p + h, :],
                        start=(jt == 0 and qt == 0),
                        stop=(jt == NJ - 1 and qt == NQT - 1),
                        skip_group_check=True,
                    )
                remaining[c] -= 1
                if remaining[c] == 0:
                    finalize(c)

            # ---- main ring loop ----
            last_chunk_start = next(
                i for i, e in enumerate(events)
                if e[0] == "s" and e[1] == NCHUNK - 1)
            for base in range(0, len(events), 3):
                grp = events[base:base + 3]
                lag_now = CTX_LAG if base < last_chunk_start else 6
                # pop ctx matmuls BEFORE this group's fills: the fills wait
                # on ACT freeing the psum ring (pool WAR) and the in-order
                # PE queue would stall (and de-ramp the p-state); the pops
                # are long-ready and keep the PE busy through the wait.
                npop = 0
                while len(pending) > lag_now and npop < 6:
                    pop_ctx()
                    npop += 1
                g = gpool.tile([128, 3, 512], F32, tag="g",
                               name=f"g{base}")
                ex = None
                for pos, ev in enumerate(grp):
                    emit_fill(g, pos, ev)
                # exp the maximal scores runs of this group
                run = None
                runs = []
                for pos, ev in enumerate(grp):
                    if ev[0] == "s":
                        if run is None:
                            run = [pos, pos + 1]
                        else:
                            run[1] = pos + 1
                    else:
                        if run is not None:
                            runs.append(run)
                        run = None
                if run is not None:
                    runs.append(run)
                if runs:
                    ex = expool.tile([128, 3, 512], F16, tag="e",
                                     name=f"ex{base}")
                for a, b_ in runs:
                    nc.scalar.activation(
                        ex[:, a:b_, :], g[:, a:b_, :],
                        mybir.ActivationFunctionType.Exp,
                        scale=1.0 / np.sqrt(HEAD_DIM),
                    )
                    if use_mask:
                        for pos in range(a, b_):
                            _, c, jt, h = grp[pos]
                            nc.vector.tensor_scalar_mul(
                                ex[:, pos, :], ex[:, pos, :],
                                em[:, jt:jt + 1],
                            )
                for pos, ev in enumerate(grp):
                    if ev[0] == "s":
                        pending.append((ex, pos, ev[1], ev[2], ev[3]))

            drain_proj()
            while pending:
                pop_ctx()

    nc.compile()
    return nc


def _get_nc(use_mask: bool, use_bias: bool):
    key = (use_mask, use_bias)
    if key not in _cache:
        _cache[key] = _build(use_mask, use_bias)
    return _cache[key]


def kernel(hidden_states, attention_mask, Wq, bq, Wk, bk, Wv, bv):
    global last_results
    hidden_states = np.asarray(hidden_states, dtype=np.float32)
    attention_mask = np.asarray(attention_mask, dtype=np.float32)
    Wq = np.asarray(Wq, dtype=np.float32)
    Wk = np.asarray(Wk, dtype=np.float32)
    Wv = np.asarray(Wv, dtype=np.float32)
    bq = np.asarray(bq, dtype=np.float32)
    bk = np.asarray(bk, dtype=np.float32)
    bv = np.asarray(bv, dtype=np.float32)

    use_mask = bool(np.any(attention_mask))
    use_bias = bool(np.any(bq) or np.any(bk) or np.any(bv))
    nc = _get_nc(use_mask, use_bias)
    KC = 7 if use_bias else 6

    in_maps = []
    for c in range(8):
        b = c // 2
        hg = c % 2
        cs = slice(hg * HPC * HEAD_DIM, (hg + 1) * HPC * HEAD_DIM)

        xT = np.zeros((KC * 128, S), dtype=np.float32)
        xT[:HIDDEN] = hidden_states[b].T
        if use_bias:
            xT[HIDDEN] = 1.0

        def wslice(W, bias):
            w = np.zeros((KC * 128, HPC * HEAD_DIM), dtype=np.float32)
            w[:HIDDEN] = W[:, cs]
            if use_bias:
                w[HIDDEN] = bias[cs]
            return w

        m = {
            "xT": xT.astype(np.float16),
            "wq": wslice(Wq, bq).astype(np.float16),
            "wk": wslice(Wk, bk).astype(np.float16),
            "wv": wslice(Wv, bv).astype(np.float16),
        }
        if use_mask:
            em = np.exp(attention_mask[b, 0, 0, :]).astype(np.float32)
            m["em"] = np.ascontiguousarray(em.reshape(NJ, 128).T)
        in_maps.append(m)

    res = run_bass_kernel_spmd(
        nc, in_maps, list(range(8)),
        trace=bool(os.environ.get("KERNEL_TRACE")),
    )
    last_results = res

    out = np.empty((B, S, HIDDEN), dtype=np.float32)
    for c in range(8):
        b = c // 2
        hg = c % 2
        r = res.results[c]["out"]  # [6, 2048, 64]
        out[b, :, hg * HPC * HEAD_DIM:(hg + 1) * HPC * HEAD_DIM] = (
            r.transpose(1, 0, 2).reshape(S, HPC * HEAD_DIM)
        )
    return out
